# revision 32
# baseline (speedup 1.0000x reference)
"""BaseGCN (4-layer GCN + mean-pool + MLP) on 8 Trainium2 NeuronCores.

Strategy: dst-sharded graph parallel with all-SBUF gathers.
  - z (aggregation inputs) kept TRANSPOSED: z^T chunk tables live in SBUF as
    [128 partitions, NEL] where partitions 32c..32c+w hold features of chunk c
    (4 chunks of the global slot space, each <= 32768 slots for int16).
  - GPSIMD ap_gather pulls per-edge columns (messages) feature-major:
    msgT [128, 2048] per "bank" (32 windows x 64 positions per chunk table).
  - PE transposes 32x128 slabs -> edge-major rows, then K=64 window-pure
    matmuls against streamed norm-folded one-hot tiles accumulate
    agg^T [w, 512] in PSUM (symmetric normalization + self loops folded into
    the one-hot values; mean-pool weights folded in for layer 4).
  - Weights/bias/PReLU applied in transposed space; z^T written per-bank to
    DRAM; AllGather exchanges z^T slices between layers.
  - Layer 4 aggregates h3 @ (W4 lw1 lw2) at width 4 directly into pooled
    logits [4, 64]; AllReduce + constant fold finishes the MLP.

GCNConv(x) = A_hat (x W) + b with A_hat = D^-1/2 A D^-1/2 + D^-1 I;
aggregation commutes with the weight matmul so we aggregate at
width min(d_in, d_out): widths 8, 8, 32, 4.
"""

import os
import numpy as np

# ---------------- problem constants (hardcoded per the contract) ----------
N = 100000
E = 1600000
B = 64
NC = 8
NPC = N // NC          # 12500 dst nodes per core
WIN = 16               # nodes per window (one-hot columns / psum sub-window)
CHUNKS = 4             # src-slot chunks (tables); chunk = src_core // 2
CCAP = 64              # max edges per (window, chunk)  -> one K=64 matmul
WPB = 32               # windows per bank (psum bank = 512 node slots)
POSB = WPB * CCAP      # positions per bank per table (2048)
D_IN = 8
DIMS = [8, 32, 64]             # h widths for layers 1..3
AGG_W = [8, 8, 32, 4]          # aggregation widths per layer
NG = B // WIN                  # graph windows for layer 4 (4)
F32 = np.float32


def _bin_windows(sizes_vec):
    """Best-fit-decreasing bins: <= WIN nodes, per-chunk edge load <= CCAP.
    sizes_vec: [n, CHUNKS] int. Returns (win_of, col_of, n_windows)."""
    n = sizes_vec.shape[0]
    tot = sizes_vec.sum(1)
    order = np.argsort(-tot, kind="stable")
    win_of = np.zeros(n, np.int32)
    col_of = np.zeros(n, np.int32)
    nb = 0
    loads = np.zeros((n, CHUNKS), np.int64)
    cnts = np.zeros(n, np.int64)
    tots = np.zeros(n, np.int64)
    for v in order:
        s = sizes_vec[v]
        ok = np.nonzero((cnts[:nb] < WIN)
                        & ((loads[:nb] + s) <= CCAP).all(1))[0]
        if len(ok):
            bid = ok[np.argmax(tots[ok])]
        else:
            bid = nb
            nb += 1
        win_of[v] = bid
        col_of[v] = cnts[bid]
        loads[bid] += s
        cnts[bid] += 1
        tots[bid] += tot[v]
    return win_of, col_of, nb


def _preprocess(x, edge_index, batch):
    src = edge_index[0].astype(np.int64)
    dst = edge_index[1].astype(np.int64)
    batch = batch.astype(np.int64)

    deg = np.bincount(dst, minlength=N).astype(F32) + 1.0
    dinv = (1.0 / np.sqrt(deg)).astype(F32)

    allsrc = np.concatenate([src, np.arange(N, dtype=np.int64)])
    alldst = np.concatenate([dst, np.arange(N, dtype=np.int64)])
    allval = np.concatenate([dinv[src] * dinv[dst], dinv * dinv]).astype(F32)

    cnt = np.maximum(np.bincount(batch, minlength=B).astype(F32), 1.0)

    chunk_of_node = (np.arange(N) // NPC) // 2          # [N] 0..3
    e_chunk = chunk_of_node[allsrc].astype(np.int64)
    e_core = (alldst // NPC).astype(np.int64)

    # ---- per-core window binning (vector caps) ----
    win_of = np.zeros(N, np.int32)
    col_of = np.zeros(N, np.int32)
    nwins = []
    for c in range(NC):
        lo, hi = c * NPC, (c + 1) * NPC
        m = (alldst >= lo) & (alldst < hi)
        sizes = np.zeros((NPC, CHUNKS), np.int64)
        np.add.at(sizes, (alldst[m] - lo, e_chunk[m]), 1)
        w_o, c_o, nw = _bin_windows(sizes)
        win_of[lo:hi] = w_o
        col_of[lo:hi] = c_o
        nwins.append(nw)
    W_CNT = int(np.ceil(max(nwins) / WPB) * WPB)
    BANKS = W_CNT // WPB
    SLOTS = W_CNT * WIN
    GSLOTS = NC * SLOTS
    NEL = GSLOTS // CHUNKS                     # slots per chunk table
    assert NEL <= 32768, NEL

    slot_of = ((np.arange(N) // NPC) * SLOTS + win_of * WIN + col_of).astype(np.int64)
    e_srcslot_loc = (slot_of[allsrc] - e_chunk * NEL).astype(np.int16)
    assert (slot_of[allsrc] - e_chunk * NEL < NEL).all()

    idx123 = np.zeros((NC, BANKS, 128, POSB // 16), np.int16)
    oh123 = np.zeros((NC, BANKS, 128, WPB * WIN * 4), np.float16)

    # ---- per-core streams ----
    for c in range(NC):
        m = e_core == c
        ew = win_of[alldst[m]].astype(np.int64)
        ec = e_chunk[m]
        ecol = col_of[alldst[m]]
        esl = e_srcslot_loc[m]
        ev = allval[m]
        # position within (window, chunk)
        key = ew * CHUNKS + ec
        order = np.argsort(key, kind="stable")
        ks = key[order]
        starts = np.searchsorted(ks, np.arange(W_CNT * CHUNKS))
        pos = np.arange(len(ks)) - starts[ks]
        assert pos.max() < CCAP
        w_s, c_s = ks // CHUNKS, ks % CHUNKS
        p = (w_s % WPB) * CCAP + pos              # position in bank-table stream
        b = w_s // WPB
        lane16 = (p % 16).astype(np.int64)
        col16 = (p // 16).astype(np.int64)
        idx123[c, b, 32 * c_s + lane16, col16] = esl[order]
        idx123[c, b, 32 * c_s + 16 + lane16, col16] = esl[order]
        J = (w_s % WPB) // 2
        oh123[c, b, CCAP * (w_s % 2) + pos,
              (J * CHUNKS + c_s) * 32 + WIN * (w_s % 2) + ecol[order]] = ev[order]

    # ---- split streams for w<=8 layers (L1, L2, L4): one gather of 1024 ----
    # idx per core; core k = 2*chunk + half owns windows [16h, 16h+16) of
    # each bank for its chunk. Table replicates chunk rows at +16.
    idxS = np.zeros((NC, BANKS, 128, 64), np.int16)
    ohS = np.zeros((NC, BANKS, 128, WPB * WIN * 4), np.float16)
    for c in range(NC):
        m = e_core == c
        ew = win_of[alldst[m]].astype(np.int64)
        ec = e_chunk[m]
        ecol = col_of[alldst[m]].astype(np.int64)
        esl = e_srcslot_loc[m]
        ev = allval[m]
        key = ew * CHUNKS + ec
        order = np.argsort(key, kind="stable")
        ks = key[order]
        starts = np.searchsorted(ks, np.arange(W_CNT * CHUNKS))
        pos = np.arange(len(ks)) - starts[ks]
        w_s, c_s = ks // CHUNKS, ks % CHUNKS
        b = w_s // WPB
        lw = w_s % WPB
        h = lw // 16
        ph = (lw % 16) * CCAP + pos              # position within (b, chunk, h)
        idxS[c, b, 16 * (2 * c_s + h) + (ph % 16), ph // 16] = esl[order]
        sg = ph // 512
        jp = (ph % 512) // 128
        q = (sg * 4 + jp) * 8 + c_s * 2 + h
        ohS[c, b, ph % 128,
            q * 32 + 16 * ((ph % 128) // 64) + ecol[order]] = ev[order]

    # ---- flat layer-4 streams: compact positions, graph-column one-hots ----
    g_of_e = batch[alldst]
    val4 = (allval / cnt[g_of_e]).astype(F32)
    S4 = 0
    percore4 = []
    for c in range(NC):
        m = e_core == c
        ec = e_chunk[m]
        order4 = np.argsort(ec, kind="stable")
        ec_s = ec[order4]
        starts4 = np.searchsorted(ec_s, np.arange(CHUNKS))
        pic = np.arange(len(ec_s)) - starts4[ec_s]   # position within chunk
        h4 = pic % 2
        ph4 = pic // 2                               # position within (chunk, half)
        percore4.append((m, order4, ec_s, h4, ph4))
        S4 = max(S4, int(np.ceil((ph4.max() + 1) / 1024)))
    idx4f = np.zeros((NC, S4, 128, 64), np.int16)
    oh4f = np.zeros((NC, S4, 128, 64 * B), np.float16)
    for c in range(NC):
        m, order4, ec_s, h4, ph4 = percore4[c]
        esl = e_srcslot_loc[m][order4].astype(np.int64)
        ev4 = val4[m][order4]
        eg = g_of_e[m][order4]
        s = ph4 // 1024
        p = ph4 % 1024
        idx4f[c, s, 16 * (2 * ec_s + h4) + (p % 16), p // 16] = esl
        sg = p // 512
        jp = (p % 512) // 128
        q = (sg * 4 + jp) * 8 + ec_s * 2 + h4
        oh4f[c, s, p % 128, q * B + eg] = ev4

    # x in chunk-table layout [128, NEL], chunk rows replicated at +16
    xtab = np.zeros((128, NEL), F32)
    xs = np.zeros((GSLOTS, D_IN), F32)
    xs[slot_of] = x
    for c in range(CHUNKS):
        xtab[32 * c:32 * c + D_IN, :] = xs[c * NEL:(c + 1) * NEL].T
        xtab[32 * c + 16:32 * c + 16 + D_IN, :] = xs[c * NEL:(c + 1) * NEL].T

    cfg = dict(W_CNT=W_CNT, BANKS=BANKS, SLOTS=SLOTS, GSLOTS=GSLOTS, NEL=NEL,
               S4=S4)
    return cfg, xtab, idx123, oh123, idxS, ohS, idx4f, oh4f


def _build_program(cfg):
    import concourse.bacc as bacc
    import concourse.tile as tile
    import concourse.bass as bass
    import concourse.mybir as mybir
    from concourse.masks import make_identity
    from contextlib import ExitStack

    dt = mybir.dt
    BANKS, SLOTS, NEL = cfg["BANKS"], cfg["SLOTS"], cfg["NEL"]
    IDXW = POSB // 16        # 128
    OHW = WPB * WIN * 4      # 2048 (fp16)

    assert BANKS == 32, BANKS  # half-split write indexing assumes 16/half
    nc = bacc.Bacc("TRN2", target_bir_lowering=False, debug=False, num_devices=NC)

    xtab_d = nc.dram_tensor("xtab", [128, NEL], dt.float32, kind="ExternalInput")
    idx123_d = nc.dram_tensor("idx123", [BANKS, 128, IDXW], dt.int16, kind="ExternalInput")
    oh123_d = nc.dram_tensor("oh123", [BANKS, 128, OHW], dt.float16, kind="ExternalInput")
    idxS_d = nc.dram_tensor("idxS", [BANKS, 128, 64], dt.int16, kind="ExternalInput")
    ohS_d = nc.dram_tensor("ohS", [BANKS, 128, OHW], dt.float16, kind="ExternalInput")
    S4 = cfg["S4"]
    idx4f_d = nc.dram_tensor("idx4f", [S4, 128, 64], dt.int16,
                             kind="ExternalInput")
    oh4f_d = nc.dram_tensor("oh4f", [S4, 128, 64 * B], dt.float16,
                            kind="ExternalInput")
    Wd = {}
    for i, (ki, ko) in enumerate([(8, 8), (8, 32), (32, 64), (64, 4)]):
        Wd[i] = nc.dram_tensor(f"W{i+1}", [ki, ko], dt.float32, kind="ExternalInput")
    bd, ad = {}, {}
    for i, d in enumerate(DIMS):
        bd[i] = nc.dram_tensor(f"b{i+1}", [d, 1], dt.float32, kind="ExternalInput")
        ad[i] = nc.dram_tensor(f"a{i+1}", [d, 1], dt.float32, kind="ExternalInput")
    cvec_d = nc.dram_tensor("cvec", [4, 1], dt.float32, kind="ExternalInput")
    out_d = nc.dram_tensor("out", [4, B], dt.float32, kind="ExternalOutput")

    AG = mybir.AluOpType

    with tile.TileContext(nc) as tc, ExitStack() as ctx:
        wpool = ctx.enter_context(tc.tile_pool(name="weights", bufs=1))
        dram = ctx.enter_context(tc.tile_pool(name="dram", bufs=1, space="DRAM"))
        sb = ctx.enter_context(tc.tile_pool(name="sb", bufs=3))
        sbB = ctx.enter_context(tc.tile_pool(name="sbB", bufs=2))
        psA = ctx.enter_context(tc.tile_pool(name="psA", bufs=2, space="PSUM"))
        psB = ctx.enter_context(tc.tile_pool(name="psB", bufs=2, space="PSUM"))
        psC = ctx.enter_context(tc.tile_pool(name="psC", bufs=1, space="PSUM"))
        psT = ctx.enter_context(tc.tile_pool(name="psT", bufs=2, space="PSUM"))
        psP = ctx.enter_context(tc.tile_pool(name="psP", bufs=1, space="PSUM"))

        table = wpool.tile([128, NEL], dt.float32, name="table")
        ident = wpool.tile([128, 128], dt.float32, name="ident")
        make_identity(nc, ident[:])

        Wt, bt, at = {}, {}, {}
        for i, (ki, ko) in enumerate([(8, 8), (8, 32), (32, 64), (64, 4)]):
            Wt[i] = wpool.tile([ki, ko], dt.float32, tag=f"w{i}", name=f"wt{i}")
            nc.sync.dma_start(Wt[i][:], Wd[i][:])
        for i, d in enumerate(DIMS):
            bt[i] = wpool.tile([d, 1], dt.float32, tag=f"b{i}", name=f"bt{i}")
            nc.sync.dma_start(bt[i][:], bd[i][:])
            at[i] = wpool.tile([d, 1], dt.float32, tag=f"a{i}", name=f"at{i}")
            nc.sync.dma_start(at[i][:], ad[i][:])
        cvt = wpool.tile([4, 1], dt.float32, name="cvt")
        nc.sync.dma_start(cvt[:], cvec_d[:])

        S2 = SLOTS // 2
        zownT = {1: dram.tile([2, 8, S2], dt.float32, name="zo1"),
                 2: dram.tile([2, 16, S2], dt.float32, name="zo2"),
                 3: dram.tile([2, 4, S2], dt.float32, name="zo3")}
        zfullT = {1: dram.tile([2, NC, 8, S2], dt.float32, name="zf1"),
                  2: dram.tile([2, NC, 16, S2], dt.float32, name="zf2"),
                  3: dram.tile([2, NC, 4, S2], dt.float32, name="zf3")}
        pool_in = dram.tile([4, B], dt.float32, name="pin")
        pool_out = dram.tile([4, B], dt.float32, name="pout")

        def agg_phase(table, idx_src, oh_src, nseg, w, body, ohw=None):
            """Gather+transpose+reduce for nseg segments; body(seg, J, c,
            lhsT_ap, oh_tile) emits one K=128 matmul per (slab, chunk)."""
            for s in range(nseg):
                idx_t = sb.tile([128, IDXW], dt.int16, tag="idx", name="idx")
                nc.sync.dma_start(idx_t[:], idx_src[s])
                if ohw is None:
                    oh_t = sb.tile([128, OHW], dt.float16, tag="oh", name="oh")
                else:
                    oh_t = sbB.tile([128, ohw], dt.float16, tag="oh4", name="oh4")
                nc.sync.dma_start(oh_t[:], oh_src[s])
                msgT = sb.tile([128, POSB], dt.float32, tag="msg", name="msg")
                nc.gpsimd.ap_gather(msgT[:], table[:], idx_t[:],
                                    channels=128, num_elems=NEL, d=1,
                                    num_idxs=POSB)
                for sg in range(4):
                    trp = psT.tile([128, 512], dt.float32, tag="trp", name="trp")
                    for jp in range(4):
                        nc.tensor.transpose(
                            trp[:, jp * 128:jp * 128 + 128],
                            msgT[:, 128 * (sg * 4 + jp):128 * (sg * 4 + jp) + 128],
                            ident[:])
                    slabs = sbB.tile([128, 512], dt.float16, tag="slabs", name="slabs")
                    nc.vector.tensor_copy(slabs[:], trp[:])
                    for jp in range(4):
                        J = sg * 4 + jp
                        for c in range(CHUNKS):
                            body(s, J, c,
                                 slabs[:, jp * 128 + 32 * c:jp * 128 + 32 * c + w],
                                 oh_t)

        def agg_phase_split(table, s0, nseg, w, body, paired=False,
                            idx_src=None, oh_src=None, ohw=None):
            """Split variant: each core gathers 1024 positions of its
            (chunk, half); body(seg, sg, jp, h, c, lhsT, oh_t). With
            paired=True the table holds packed fp16 feature-pairs (w rows
            of 2 features); lhsT slices are 2*w fp16 columns."""
            for s in range(s0, s0 + nseg):
                idx_t = sb.tile([128, 64], dt.int16, tag="idxS", name="idxS")
                nc.sync.dma_start(idx_t[:], (idx_src or idxS_d)[s])
                if ohw is None:
                    oh_t = sb.tile([128, OHW], dt.float16, tag="oh", name="oh")
                else:
                    oh_t = sbB.tile([128, ohw], dt.float16, tag="oh4", name="oh4")
                nc.sync.dma_start(oh_t[:], (oh_src or ohS_d)[s])
                msgT = sb.tile([128, 1024], dt.float32, tag="msgS", name="msgS")
                nc.gpsimd.ap_gather(msgT[:], table[:], idx_t[:],
                                    channels=128, num_elems=NEL, d=1,
                                    num_idxs=1024)
                for sg in range(2):
                    trp = psT.tile([128, 512], dt.float32, tag="trp", name="trp")
                    for jp in range(4):
                        nc.tensor.transpose(
                            trp[:, jp * 128:jp * 128 + 128],
                            msgT[:, 512 * sg + 128 * jp:512 * sg + 128 * jp + 128],
                            ident[:])
                    if paired:
                        slabs = sbB.tile([128, 1024], dt.float16, tag="slabsP",
                                         name="slabsP")
                        nc.vector.tensor_copy(slabs[:].bitcast(dt.float32),
                                              trp[:])
                    else:
                        slabs = sbB.tile([128, 512], dt.float16, tag="slabs",
                                         name="slabs")
                        nc.vector.tensor_copy(slabs[:], trp[:])
                    for jp in range(4):
                        for h in range(2):
                            for c in range(CHUNKS):
                                off = jp * 128 + 32 * c + 16 * h
                                if paired:
                                    lhs = slabs[:, 2 * off:2 * off + 2 * w]
                                else:
                                    lhs = slabs[:, off:off + w]
                                body(s, sg, jp, h, c, lhs, oh_t)

        def load_table_split(zf, w):
            for c in range(CHUNKS):
                for rep in (0, 16):
                    for hb in (0, 1):
                        nc.scalar.dma_start(
                            table[32 * c + rep:32 * c + rep + w,
                                  hb * S2:hb * S2 + S2], zf[hb][2 * c])
                        nc.scalar.dma_start(
                            table[32 * c + rep:32 * c + rep + w,
                                  SLOTS + hb * S2:SLOTS + hb * S2 + S2],
                            zf[hb][2 * c + 1])

        def layer(l):  # l = 0, 1, 2
            w = AGG_W[l]
            d = DIMS[l]
            if l == 0:
                nc.scalar.dma_start(table[:], xtab_d[:])
            elif l == 1:
                load_table_split(zfullT[l], w)
            else:
                load_table_split(zfullT[l], 16)   # fp16 feature-pairs

            state = {}

            def body(bank, J, c, lhsT, oh_t):
                if J == 0 and c == 0:
                    state["agg"] = psA.tile([w, 512], dt.float32, tag="agg",
                                            name="agg")
                nc.tensor.matmul(state["agg"][:, 32 * J:32 * J + 32],
                                 lhsT=lhsT,
                                 rhs=oh_t[:, (J * 4 + c) * 32:(J * 4 + c) * 32 + 32],
                                 start=(c == 0), stop=(c == CHUNKS - 1))
                if J == WPB // 2 - 1 and c == CHUNKS - 1:
                    bphase(bank, state["agg"])

            def bodyS(bank, sg, jp, h, c, lhsT, oh_t):
                if sg == 0 and jp == 0 and h == 0 and c == 0:
                    state["agg"] = psA.tile([w, 512], dt.float32, tag="agg",
                                            name="agg")
                q = (sg * 4 + jp) * 8 + c * 2 + h
                nc.tensor.matmul(
                    state["agg"][:, 256 * h + 32 * (sg * 4 + jp):
                                 256 * h + 32 * (sg * 4 + jp) + 32],
                    lhsT=lhsT, rhs=oh_t[:, q * 32:q * 32 + 32],
                    start=(c == 0), stop=(c == CHUNKS - 1))
                if sg == 1 and jp == 3 and h == 1 and c == CHUNKS - 1:
                    bphase(bank, state["agg"])

            def bphase(bank, agg_ps):
                aggs = sbB.tile([w, 512], dt.float32, tag="aggs", name="aggs")
                nc.vector.tensor_copy(aggs[:], agg_ps[:])
                h_ps = psB.tile([d, 512], dt.float32, tag="h", name="h")
                nc.tensor.matmul(h_ps[:], lhsT=Wt[l][:], rhs=aggs[:],
                                 start=True, stop=True)
                neg = sbB.tile([d, 512], dt.float32, tag="neg", name="neg")
                nc.vector.tensor_scalar(neg[:], h_ps[:], bt[l][:], 0.0, AG.add, AG.min)
                nega = sbB.tile([d, 512], dt.float32, tag="nega", name="nega")
                nc.vector.tensor_scalar(nega[:], neg[:], at[l][:], None, AG.mult)
                pos = sbB.tile([d, 512], dt.float32, tag="pos", name="pos")
                nc.vector.tensor_scalar(pos[:], h_ps[:], bt[l][:], 0.0, AG.add, AG.max)
                hT = sbB.tile([d, 512], dt.float32, tag="hT", name="hT")
                nc.vector.tensor_add(hT[:], pos[:], nega[:])
                if l == 2:
                    z4_ps = psC.tile([4, 512], dt.float32, tag="z4", name="z4")
                    nc.tensor.matmul(z4_ps[:], lhsT=Wt[3][:], rhs=hT[:],
                                     start=True, stop=True)
                    z4s = sbB.tile([4, 512], dt.float32, tag="z4s", name="z4s")
                    nc.vector.tensor_copy(z4s[:], z4_ps[:])
                    nc.scalar.dma_start(zownT[3][bank // 16][:, 512 * (bank % 16):512 * (bank % 16) + 512], z4s[:])
                elif l == 1:
                    # pack h2 [32, 512] into fp16 feature-pairs [16, 512]
                    trp2 = psT.tile([128, 512], dt.float32, tag="trp", name="trp2")
                    for j in range(4):
                        nc.tensor.transpose(trp2[:, 32 * j:32 * j + 32],
                                            hT[:, 128 * j:128 * j + 128],
                                            ident[0:32, 0:32])
                    pk16 = sbB.tile([128, 128], dt.float16, tag="pk16", name="pk16")
                    nc.vector.tensor_copy(pk16[:], trp2[:, 0:128])
                    packT = psB.tile([16, 512], dt.float32, tag="h", name="packT")
                    for j in range(4):
                        nc.tensor.transpose(
                            packT[:, 128 * j:128 * j + 128],
                            pk16[:, 32 * j:32 * j + 32].bitcast(dt.float32),
                            ident[:])
                    packs = sbB.tile([16, 512], dt.float32, tag="hT", name="packs")
                    nc.vector.tensor_copy(packs[:], packT[:])
                    nc.scalar.dma_start(
                        zownT[2][bank // 16][:, 512 * (bank % 16):
                                             512 * (bank % 16) + 512], packs[:])
                else:
                    nc.scalar.dma_start(
                        zownT[l + 1][bank // 16][:, 512 * (bank % 16):
                                                 512 * (bank % 16) + 512], hT[:])

            zkey = l + 1 if l < 2 else 3
            wt = 16 if l == 2 else w

            def emit_ag(hb):
                if os.environ.get("GCN_NO_CC"):
                    nc.sync.dma_start(zfullT[zkey][hb][0], zownT[zkey][hb])
                else:
                    nc.gpsimd.collective_compute(
                        "AllGather", AG.bypass,
                        replica_groups=[list(range(NC))],
                        ins=[zownT[zkey][hb].opt()],
                        outs=[zfullT[zkey][hb].opt()])

            agg_phase_split(table, 0, BANKS // 2, wt, bodyS, paired=(l == 2))
            emit_ag(0)
            agg_phase_split(table, BANKS // 2, BANKS - BANKS // 2, wt, bodyS,
                            paired=(l == 2))
            emit_ag(1)

        for l in range(3):
            layer(l)

        # ---- layer 4: flat edge streams scatter straight into pooled [4, B] ----
        load_table_split(zfullT[3], 4)
        pool_ps = psP.tile([4, B], dt.float32, name="pool_ps")

        def body4(s, sg, jp, h, c, lhsT, oh_t):
            q = (sg * 4 + jp) * 8 + c * 2 + h
            nc.tensor.matmul(
                pool_ps[:], lhsT=lhsT, rhs=oh_t[:, q * B:q * B + B],
                start=(s == 0 and sg == 0 and jp == 0 and h == 0 and c == 0),
                stop=(s == S4 - 1 and sg == 1 and jp == 3 and h == 1
                      and c == CHUNKS - 1))

        agg_phase_split(table, 0, S4, 4, body4,
                        idx_src=idx4f_d, oh_src=oh4f_d, ohw=64 * B)

        pooledT = sbB.tile([4, B], dt.float32, name="pooledT")
        nc.vector.tensor_copy(pooledT[:], pool_ps[:])
        nc.sync.dma_start(pool_in[:], pooledT[:])
        if os.environ.get("GCN_NO_CC"):
            nc.sync.dma_start(pool_out[:], pool_in[:])
        else:
            nc.gpsimd.collective_compute(
                "AllReduce", AG.add, replica_groups=[list(range(NC))],
                ins=[pool_in[:].opt()], outs=[pool_out[:].opt()])
        res = sbB.tile([4, B], dt.float32, name="res")
        nc.sync.dma_start(res[:], pool_out[:])
        res2 = sbB.tile([4, B], dt.float32, name="res2")
        nc.vector.tensor_scalar(res2[:], res[:], cvt[:], None, AG.add)
        nc.sync.dma_start(out_d[:], res2[:])

    nc.compile()
    return nc


def build(inputs):
    """Host preprocessing + program build. Returns (nc, in_maps)."""
    x = np.asarray(inputs["x"], F32)
    edge_index = np.asarray(inputs["edge_index"])
    batch = np.asarray(inputs["batch"])
    W = [np.asarray(inputs[f"W{i}"], F32) for i in range(1, 5)]
    b = [np.asarray(inputs[f"b{i}"], F32) for i in range(1, 5)]
    a = [np.asarray(inputs[f"a{i}"], F32) for i in range(1, 4)]
    lw1 = np.asarray(inputs["lw1"], F32)
    lb1 = np.asarray(inputs["lb1"], F32)
    lw2 = np.asarray(inputs["lw2"], F32)
    lb2 = np.asarray(inputs["lb2"], F32)

    cfg, xtab, idx123, oh123, idxS, ohS, idx4f, oh4f = _preprocess(x, edge_index, batch)

    W4p = (W[3] @ lw1 @ lw2).astype(F32)                     # [64, 4]
    cv = (b[3] @ lw1 @ lw2 + lb1 @ lw2 + lb2).astype(F32)    # [4]

    nc = _build_program(cfg)

    in_maps = []
    for c in range(NC):
        m = dict(
            xtab=xtab, idx123=idx123[c], oh123=oh123[c],
            idxS=idxS[c], ohS=ohS[c], idx4f=idx4f[c], oh4f=oh4f[c],
            W1=W[0], W2=W[1], W3=W[2], W4=W4p,
            b1=b[0].reshape(-1, 1), b2=b[1].reshape(-1, 1), b3=b[2].reshape(-1, 1),
            a1=np.full((8, 1), a[0][0], F32),
            a2=np.full((32, 1), a[1][0], F32),
            a3=np.full((64, 1), a[2][0], F32),
            cvec=cv.reshape(4, 1),
        )
        in_maps.append(m)
    return nc, in_maps


def kernel(**inputs):
    nc, in_maps = build(inputs)
    from concourse.bass_utils import run_bass_kernel_spmd
    res = run_bass_kernel_spmd(nc, in_maps, list(range(NC)))
    outT = res.results[0]["out"]      # [4, B]
    return np.ascontiguousarray(outT.T.astype(F32))          # [B, 4]



# revision 33
# speedup vs baseline: 1.1753x; 1.1753x over previous
"""BaseGCN (4-layer GCN + mean-pool + MLP) on 8 Trainium2 NeuronCores.

Strategy: dst-sharded graph parallel with all-SBUF gathers.
  - z (aggregation inputs) kept TRANSPOSED: z^T chunk tables live in SBUF as
    [128 partitions, NEL] where partitions 32c..32c+w hold features of chunk c
    (4 chunks of the global slot space, each <= 32768 slots for int16).
  - GPSIMD ap_gather pulls per-edge columns (messages) feature-major:
    msgT [128, 2048] per "bank" (32 windows x 64 positions per chunk table).
  - PE transposes 32x128 slabs -> edge-major rows, then K=64 window-pure
    matmuls against streamed norm-folded one-hot tiles accumulate
    agg^T [w, 512] in PSUM (symmetric normalization + self loops folded into
    the one-hot values; mean-pool weights folded in for layer 4).
  - Weights/bias/PReLU applied in transposed space; z^T written per-bank to
    DRAM; AllGather exchanges z^T slices between layers.
  - Layer 4 aggregates h3 @ (W4 lw1 lw2) at width 4 directly into pooled
    logits [4, 64]; AllReduce + constant fold finishes the MLP.

GCNConv(x) = A_hat (x W) + b with A_hat = D^-1/2 A D^-1/2 + D^-1 I;
aggregation commutes with the weight matmul so we aggregate at
width min(d_in, d_out): widths 8, 8, 32, 4.
"""

import os
import numpy as np

# ---------------- problem constants (hardcoded per the contract) ----------
N = 100000
E = 1600000
B = 64
NC = 8
NPC = N // NC          # 12500 dst nodes per core
WIN = 16               # nodes per window (one-hot columns / psum sub-window)
CHUNKS = 4             # src-slot chunks (tables); chunk = src_core // 2
CCAP = 64              # max edges per (window, chunk)  -> one K=64 matmul
WPB = 32               # windows per bank (psum bank = 512 node slots)
POSB = WPB * CCAP      # positions per bank per table (2048)
D_IN = 8
DIMS = [8, 32, 64]             # h widths for layers 1..3
AGG_W = [8, 8, 32, 4]          # aggregation widths per layer
NG = B // WIN                  # graph windows for layer 4 (4)
F32 = np.float32


def _bin_windows(sizes_vec):
    """Best-fit-decreasing bins: <= WIN nodes, per-chunk edge load <= CCAP.
    sizes_vec: [n, CHUNKS] int. Returns (win_of, col_of, n_windows)."""
    n = sizes_vec.shape[0]
    tot = sizes_vec.sum(1)
    order = np.argsort(-tot, kind="stable")
    win_of = np.zeros(n, np.int32)
    col_of = np.zeros(n, np.int32)
    nb = 0
    loads = np.zeros((n, CHUNKS), np.int64)
    cnts = np.zeros(n, np.int64)
    tots = np.zeros(n, np.int64)
    for v in order:
        s = sizes_vec[v]
        ok = np.nonzero((cnts[:nb] < WIN)
                        & ((loads[:nb] + s) <= CCAP).all(1))[0]
        if len(ok):
            bid = ok[np.argmax(tots[ok])]
        else:
            bid = nb
            nb += 1
        win_of[v] = bid
        col_of[v] = cnts[bid]
        loads[bid] += s
        cnts[bid] += 1
        tots[bid] += tot[v]
    return win_of, col_of, nb


def _preprocess(x, edge_index, batch):
    src = edge_index[0].astype(np.int64)
    dst = edge_index[1].astype(np.int64)
    batch = batch.astype(np.int64)

    deg = np.bincount(dst, minlength=N).astype(F32) + 1.0
    dinv = (1.0 / np.sqrt(deg)).astype(F32)

    allsrc = np.concatenate([src, np.arange(N, dtype=np.int64)])
    alldst = np.concatenate([dst, np.arange(N, dtype=np.int64)])
    allval = np.concatenate([dinv[src] * dinv[dst], dinv * dinv]).astype(F32)

    cnt = np.maximum(np.bincount(batch, minlength=B).astype(F32), 1.0)

    chunk_of_node = (np.arange(N) // NPC) // 2          # [N] 0..3
    e_chunk = chunk_of_node[allsrc].astype(np.int64)
    e_core = (alldst // NPC).astype(np.int64)

    # ---- per-core window binning (vector caps) ----
    win_of = np.zeros(N, np.int32)
    col_of = np.zeros(N, np.int32)
    nwins = []
    for c in range(NC):
        lo, hi = c * NPC, (c + 1) * NPC
        m = (alldst >= lo) & (alldst < hi)
        sizes = np.zeros((NPC, CHUNKS), np.int64)
        np.add.at(sizes, (alldst[m] - lo, e_chunk[m]), 1)
        w_o, c_o, nw = _bin_windows(sizes)
        win_of[lo:hi] = w_o
        col_of[lo:hi] = c_o
        nwins.append(nw)
    W_CNT = int(np.ceil(max(nwins) / WPB) * WPB)
    BANKS = W_CNT // WPB
    SLOTS = W_CNT * WIN
    GSLOTS = NC * SLOTS
    NEL = GSLOTS // CHUNKS                     # slots per chunk table
    assert NEL <= 32768, NEL

    slot_of = ((np.arange(N) // NPC) * SLOTS + win_of * WIN + col_of).astype(np.int64)
    e_srcslot_loc = (slot_of[allsrc] - e_chunk * NEL).astype(np.int16)
    assert (slot_of[allsrc] - e_chunk * NEL < NEL).all()

    idx123 = np.zeros((NC, BANKS, 128, POSB // 16), np.int16)
    oh123 = np.zeros((NC, BANKS, 128, WPB * WIN * 4), np.float16)

    # ---- per-core streams ----
    for c in range(NC):
        m = e_core == c
        ew = win_of[alldst[m]].astype(np.int64)
        ec = e_chunk[m]
        ecol = col_of[alldst[m]]
        esl = e_srcslot_loc[m]
        ev = allval[m]
        # position within (window, chunk)
        key = ew * CHUNKS + ec
        order = np.argsort(key, kind="stable")
        ks = key[order]
        starts = np.searchsorted(ks, np.arange(W_CNT * CHUNKS))
        pos = np.arange(len(ks)) - starts[ks]
        assert pos.max() < CCAP
        w_s, c_s = ks // CHUNKS, ks % CHUNKS
        p = (w_s % WPB) * CCAP + pos              # position in bank-table stream
        b = w_s // WPB
        lane16 = (p % 16).astype(np.int64)
        col16 = (p // 16).astype(np.int64)
        idx123[c, b, 32 * c_s + lane16, col16] = esl[order]
        idx123[c, b, 32 * c_s + 16 + lane16, col16] = esl[order]
        J = (w_s % WPB) // 2
        oh123[c, b, CCAP * (w_s % 2) + pos,
              (J * CHUNKS + c_s) * 32 + WIN * (w_s % 2) + ecol[order]] = ev[order]

    # ---- split streams for w<=8 layers (L1, L2, L4): one gather of 1024 ----
    # idx per core; core k = 2*chunk + half owns windows [16h, 16h+16) of
    # each bank for its chunk. Table replicates chunk rows at +16.
    idxS = np.zeros((NC, BANKS, 128, 64), np.int16)
    ohS = np.zeros((NC, BANKS, 128, WPB * WIN * 4), np.float16)
    for c in range(NC):
        m = e_core == c
        ew = win_of[alldst[m]].astype(np.int64)
        ec = e_chunk[m]
        ecol = col_of[alldst[m]].astype(np.int64)
        esl = e_srcslot_loc[m]
        ev = allval[m]
        key = ew * CHUNKS + ec
        order = np.argsort(key, kind="stable")
        ks = key[order]
        starts = np.searchsorted(ks, np.arange(W_CNT * CHUNKS))
        pos = np.arange(len(ks)) - starts[ks]
        w_s, c_s = ks // CHUNKS, ks % CHUNKS
        b = w_s // WPB
        lw = w_s % WPB
        h = lw // 16
        ph = (lw % 16) * CCAP + pos              # position within (b, chunk, h)
        idxS[c, b, 16 * (2 * c_s + h) + (ph % 16), ph // 16] = esl[order]
        sg = ph // 512
        jp = (ph % 512) // 128
        q = (sg * 4 + jp) * 8 + c_s * 2 + h
        ohS[c, b, ph % 128,
            q * 32 + 16 * ((ph % 128) // 64) + ecol[order]] = ev[order]

    # ---- pool one-hot: per-core [BANKS, 128, 4*B] fp16, slot -> graph/cnt ----
    pooltab = np.zeros((NC, BANKS, 128, 4 * B), np.float16)
    nodes = np.arange(N)
    core_of = nodes // NPC
    lslot = win_of * WIN + col_of                # local slot within core
    g_of_n = batch[nodes]
    pb = lslot // 512
    pj = (lslot % 512) // 128
    pp = lslot % 128
    pooltab[core_of, pb, pp, pj * B + g_of_n] = (1.0 / cnt[g_of_n]).astype(np.float16)

    # x in chunk-table layout [128, NEL], chunk rows replicated at +16
    xtab = np.zeros((128, NEL), F32)
    xs = np.zeros((GSLOTS, D_IN), F32)
    xs[slot_of] = x
    for c in range(CHUNKS):
        xtab[32 * c:32 * c + D_IN, :] = xs[c * NEL:(c + 1) * NEL].T
        xtab[32 * c + 16:32 * c + 16 + D_IN, :] = xs[c * NEL:(c + 1) * NEL].T

    cfg = dict(W_CNT=W_CNT, BANKS=BANKS, SLOTS=SLOTS, GSLOTS=GSLOTS, NEL=NEL)
    return cfg, xtab, idx123, oh123, idxS, ohS, pooltab


def _build_program(cfg):
    import concourse.bacc as bacc
    import concourse.tile as tile
    import concourse.bass as bass
    import concourse.mybir as mybir
    from concourse.masks import make_identity
    from contextlib import ExitStack

    dt = mybir.dt
    BANKS, SLOTS, NEL = cfg["BANKS"], cfg["SLOTS"], cfg["NEL"]
    IDXW = POSB // 16        # 128
    OHW = WPB * WIN * 4      # 2048 (fp16)

    assert BANKS == 32, BANKS  # half-split write indexing assumes 16/half
    nc = bacc.Bacc("TRN2", target_bir_lowering=False, debug=False, num_devices=NC)

    xtab_d = nc.dram_tensor("xtab", [128, NEL], dt.float32, kind="ExternalInput")
    idx123_d = nc.dram_tensor("idx123", [BANKS, 128, IDXW], dt.int16, kind="ExternalInput")
    oh123_d = nc.dram_tensor("oh123", [BANKS, 128, OHW], dt.float16, kind="ExternalInput")
    idxS_d = nc.dram_tensor("idxS", [BANKS, 128, 64], dt.int16, kind="ExternalInput")
    ohS_d = nc.dram_tensor("ohS", [BANKS, 128, OHW], dt.float16, kind="ExternalInput")
    ptab_d = nc.dram_tensor("pooltab", [BANKS, 128, 4 * B], dt.float16,
                            kind="ExternalInput")
    Wd = {}
    for i, (ki, ko) in enumerate([(8, 8), (8, 32), (32, 64), (64, 4)]):
        Wd[i] = nc.dram_tensor(f"W{i+1}", [ki, ko], dt.float32, kind="ExternalInput")
    bd, ad = {}, {}
    for i, d in enumerate(DIMS):
        bd[i] = nc.dram_tensor(f"b{i+1}", [d, 1], dt.float32, kind="ExternalInput")
        ad[i] = nc.dram_tensor(f"a{i+1}", [d, 1], dt.float32, kind="ExternalInput")
    cvec_d = nc.dram_tensor("cvec", [4, 1], dt.float32, kind="ExternalInput")
    out_d = nc.dram_tensor("out", [4, B], dt.float32, kind="ExternalOutput")

    AG = mybir.AluOpType

    with tile.TileContext(nc) as tc, ExitStack() as ctx:
        wpool = ctx.enter_context(tc.tile_pool(name="weights", bufs=1))
        dram = ctx.enter_context(tc.tile_pool(name="dram", bufs=1, space="DRAM"))
        sb = ctx.enter_context(tc.tile_pool(name="sb", bufs=3))
        sbB = ctx.enter_context(tc.tile_pool(name="sbB", bufs=2))
        psA = ctx.enter_context(tc.tile_pool(name="psA", bufs=2, space="PSUM"))
        psB = ctx.enter_context(tc.tile_pool(name="psB", bufs=2, space="PSUM"))
        psC = ctx.enter_context(tc.tile_pool(name="psC", bufs=1, space="PSUM"))
        psT = ctx.enter_context(tc.tile_pool(name="psT", bufs=2, space="PSUM"))
        psP = ctx.enter_context(tc.tile_pool(name="psP", bufs=1, space="PSUM"))

        table = wpool.tile([128, NEL], dt.float32, name="table")
        ident = wpool.tile([128, 128], dt.float32, name="ident")
        make_identity(nc, ident[:])

        Wt, bt, at = {}, {}, {}
        for i, (ki, ko) in enumerate([(8, 8), (8, 32), (32, 64), (64, 4)]):
            Wt[i] = wpool.tile([ki, ko], dt.float32, tag=f"w{i}", name=f"wt{i}")
            nc.sync.dma_start(Wt[i][:], Wd[i][:])
        for i, d in enumerate(DIMS):
            bt[i] = wpool.tile([d, 1], dt.float32, tag=f"b{i}", name=f"bt{i}")
            nc.sync.dma_start(bt[i][:], bd[i][:])
            at[i] = wpool.tile([d, 1], dt.float32, tag=f"a{i}", name=f"at{i}")
            nc.sync.dma_start(at[i][:], ad[i][:])
        cvt = wpool.tile([4, 1], dt.float32, name="cvt")
        nc.sync.dma_start(cvt[:], cvec_d[:])

        S2 = SLOTS // 2
        zownT = {1: dram.tile([2, 8, S2], dt.float32, name="zo1"),
                 2: dram.tile([2, 16, S2], dt.float32, name="zo2"),
                 3: dram.tile([2, 4, S2], dt.float32, name="zo3")}
        zfullT = {1: dram.tile([2, NC, 8, S2], dt.float32, name="zf1"),
                  2: dram.tile([2, NC, 16, S2], dt.float32, name="zf2"),
                  3: dram.tile([2, NC, 4, S2], dt.float32, name="zf3")}
        pool_in = dram.tile([4, B], dt.float32, name="pin")
        pool_out = dram.tile([4, B], dt.float32, name="pout")

        def agg_phase(table, idx_src, oh_src, nseg, w, body, ohw=None):
            """Gather+transpose+reduce for nseg segments; body(seg, J, c,
            lhsT_ap, oh_tile) emits one K=128 matmul per (slab, chunk)."""
            for s in range(nseg):
                idx_t = sb.tile([128, IDXW], dt.int16, tag="idx", name="idx")
                nc.sync.dma_start(idx_t[:], idx_src[s])
                oh_t = sb.tile([128, ohw or OHW], dt.float16, tag="oh", name="oh")
                nc.sync.dma_start(oh_t[:], oh_src[s])
                msgT = sb.tile([128, POSB], dt.float32, tag="msg", name="msg")
                nc.gpsimd.ap_gather(msgT[:], table[:], idx_t[:],
                                    channels=128, num_elems=NEL, d=1,
                                    num_idxs=POSB)
                for sg in range(4):
                    trp = psT.tile([128, 512], dt.float32, tag="trp", name="trp")
                    for jp in range(4):
                        nc.tensor.transpose(
                            trp[:, jp * 128:jp * 128 + 128],
                            msgT[:, 128 * (sg * 4 + jp):128 * (sg * 4 + jp) + 128],
                            ident[:])
                    slabs = sbB.tile([128, 512], dt.float16, tag="slabs", name="slabs")
                    nc.vector.tensor_copy(slabs[:], trp[:])
                    for jp in range(4):
                        J = sg * 4 + jp
                        for c in range(CHUNKS):
                            body(s, J, c,
                                 slabs[:, jp * 128 + 32 * c:jp * 128 + 32 * c + w],
                                 oh_t)

        def agg_phase_split(table, s0, nseg, w, body, paired=False):
            """Split variant: each core gathers 1024 positions of its
            (chunk, half); body(seg, sg, jp, h, c, lhsT, oh_t). With
            paired=True the table holds packed fp16 feature-pairs (w rows
            of 2 features); lhsT slices are 2*w fp16 columns."""
            for s in range(s0, s0 + nseg):
                idx_t = sb.tile([128, 64], dt.int16, tag="idxS", name="idxS")
                nc.sync.dma_start(idx_t[:], idxS_d[s])
                oh_t = sb.tile([128, OHW], dt.float16, tag="oh", name="oh")
                nc.sync.dma_start(oh_t[:], ohS_d[s])
                msgT = sb.tile([128, 1024], dt.float32, tag="msgS", name="msgS")
                nc.gpsimd.ap_gather(msgT[:], table[:], idx_t[:],
                                    channels=128, num_elems=NEL, d=1,
                                    num_idxs=1024)
                for sg in range(2):
                    trp = psT.tile([128, 512], dt.float32, tag="trp", name="trp")
                    for jp in range(4):
                        nc.tensor.transpose(
                            trp[:, jp * 128:jp * 128 + 128],
                            msgT[:, 512 * sg + 128 * jp:512 * sg + 128 * jp + 128],
                            ident[:])
                    if paired:
                        slabs = sbB.tile([128, 1024], dt.float16, tag="slabsP",
                                         name="slabsP")
                        nc.vector.tensor_copy(slabs[:].bitcast(dt.float32),
                                              trp[:])
                    else:
                        slabs = sbB.tile([128, 512], dt.float16, tag="slabs",
                                         name="slabs")
                        nc.vector.tensor_copy(slabs[:], trp[:])
                    for jp in range(4):
                        for h in range(2):
                            for c in range(CHUNKS):
                                off = jp * 128 + 32 * c + 16 * h
                                if paired:
                                    lhs = slabs[:, 2 * off:2 * off + 2 * w]
                                else:
                                    lhs = slabs[:, off:off + w]
                                body(s, sg, jp, h, c, lhs, oh_t)

        def load_table_split(zf, w):
            for c in range(CHUNKS):
                for rep in (0, 16):
                    for hb in (0, 1):
                        nc.scalar.dma_start(
                            table[32 * c + rep:32 * c + rep + w,
                                  hb * S2:hb * S2 + S2], zf[hb][2 * c])
                        nc.scalar.dma_start(
                            table[32 * c + rep:32 * c + rep + w,
                                  SLOTS + hb * S2:SLOTS + hb * S2 + S2],
                            zf[hb][2 * c + 1])

        def layer(l):  # l = 0, 1, 2
            w = AGG_W[l]
            d = DIMS[l]
            if l == 0:
                nc.scalar.dma_start(table[:], xtab_d[:])
            elif l == 1:
                load_table_split(zfullT[l], w)
            else:
                load_table_split(zfullT[l], 16)   # fp16 feature-pairs

            state = {}

            def body(bank, J, c, lhsT, oh_t):
                if J == 0 and c == 0:
                    state["agg"] = psA.tile([w, 512], dt.float32, tag="agg",
                                            name="agg")
                nc.tensor.matmul(state["agg"][:, 32 * J:32 * J + 32],
                                 lhsT=lhsT,
                                 rhs=oh_t[:, (J * 4 + c) * 32:(J * 4 + c) * 32 + 32],
                                 start=(c == 0), stop=(c == CHUNKS - 1))
                if J == WPB // 2 - 1 and c == CHUNKS - 1:
                    bphase(bank, state["agg"])

            def bodyS(bank, sg, jp, h, c, lhsT, oh_t):
                if sg == 0 and jp == 0 and h == 0 and c == 0:
                    state["agg"] = psA.tile([w, 512], dt.float32, tag="agg",
                                            name="agg")
                q = (sg * 4 + jp) * 8 + c * 2 + h
                nc.tensor.matmul(
                    state["agg"][:, 256 * h + 32 * (sg * 4 + jp):
                                 256 * h + 32 * (sg * 4 + jp) + 32],
                    lhsT=lhsT, rhs=oh_t[:, q * 32:q * 32 + 32],
                    start=(c == 0), stop=(c == CHUNKS - 1))
                if sg == 1 and jp == 3 and h == 1 and c == CHUNKS - 1:
                    bphase(bank, state["agg"])

            def bphase(bank, agg_ps):
                aggs = sbB.tile([w, 512], dt.float32, tag="aggs", name="aggs")
                nc.vector.tensor_copy(aggs[:], agg_ps[:])
                h_ps = psB.tile([d, 512], dt.float32, tag="h", name="h")
                nc.tensor.matmul(h_ps[:], lhsT=Wt[l][:], rhs=aggs[:],
                                 start=True, stop=True)
                neg = sbB.tile([d, 512], dt.float32, tag="neg", name="neg")
                nc.vector.tensor_scalar(neg[:], h_ps[:], bt[l][:], 0.0, AG.add, AG.min)
                nega = sbB.tile([d, 512], dt.float32, tag="nega", name="nega")
                nc.vector.tensor_scalar(nega[:], neg[:], at[l][:], None, AG.mult)
                pos = sbB.tile([d, 512], dt.float32, tag="pos", name="pos")
                nc.vector.tensor_scalar(pos[:], h_ps[:], bt[l][:], 0.0, AG.add, AG.max)
                hT = sbB.tile([d, 512], dt.float32, tag="hT", name="hT")
                nc.vector.tensor_add(hT[:], pos[:], nega[:])
                if l == 2:
                    z4_ps = psC.tile([4, 512], dt.float32, tag="z4", name="z4")
                    nc.tensor.matmul(z4_ps[:], lhsT=Wt[3][:], rhs=hT[:],
                                     start=True, stop=True)
                    z4s = sbB.tile([4, 512], dt.float32, tag="z4s", name="z4s")
                    nc.vector.tensor_copy(z4s[:], z4_ps[:])
                    nc.scalar.dma_start(zownT[3][bank // 16][:, 512 * (bank % 16):512 * (bank % 16) + 512], z4s[:])
                elif l == 1:
                    # pack h2 [32, 512] into fp16 feature-pairs [16, 512]
                    trp2 = psT.tile([128, 512], dt.float32, tag="trp", name="trp2")
                    for j in range(4):
                        nc.tensor.transpose(trp2[:, 32 * j:32 * j + 32],
                                            hT[:, 128 * j:128 * j + 128],
                                            ident[0:32, 0:32])
                    pk16 = sbB.tile([128, 128], dt.float16, tag="pk16", name="pk16")
                    nc.vector.tensor_copy(pk16[:], trp2[:, 0:128])
                    packT = psB.tile([16, 512], dt.float32, tag="h", name="packT")
                    for j in range(4):
                        nc.tensor.transpose(
                            packT[:, 128 * j:128 * j + 128],
                            pk16[:, 32 * j:32 * j + 32].bitcast(dt.float32),
                            ident[:])
                    packs = sbB.tile([16, 512], dt.float32, tag="hT", name="packs")
                    nc.vector.tensor_copy(packs[:], packT[:])
                    nc.scalar.dma_start(
                        zownT[2][bank // 16][:, 512 * (bank % 16):
                                             512 * (bank % 16) + 512], packs[:])
                else:
                    nc.scalar.dma_start(
                        zownT[l + 1][bank // 16][:, 512 * (bank % 16):
                                                 512 * (bank % 16) + 512], hT[:])

            zkey = l + 1 if l < 2 else 3
            wt = 16 if l == 2 else w

            def emit_ag(hb):
                if os.environ.get("GCN_NO_CC"):
                    nc.sync.dma_start(zfullT[zkey][hb][0], zownT[zkey][hb])
                else:
                    nc.gpsimd.collective_compute(
                        "AllGather", AG.bypass,
                        replica_groups=[list(range(NC))],
                        ins=[zownT[zkey][hb].opt()],
                        outs=[zfullT[zkey][hb].opt()])

            agg_phase_split(table, 0, BANKS // 2, wt, bodyS, paired=(l == 2))
            emit_ag(0)
            agg_phase_split(table, BANKS // 2, BANKS - BANKS // 2, wt, bodyS,
                            paired=(l == 2))
            emit_ag(1)

        for l in range(3):
            layer(l)

        # ---- layer 4: same aggregation tables at width 4, pool per bank ----
        load_table_split(zfullT[3], 4)
        pool_ps = psP.tile([B, 4], dt.float32, name="pool_ps")
        st4 = {}

        def body4(bank, sg, jp, h, c, lhsT, oh_t):
            if sg == 0 and jp == 0 and h == 0 and c == 0:
                st4["agg"] = psA.tile([4, 512], dt.float32, tag="agg", name="agg")
            q = (sg * 4 + jp) * 8 + c * 2 + h
            nc.tensor.matmul(
                st4["agg"][:, 256 * h + 32 * (sg * 4 + jp):
                           256 * h + 32 * (sg * 4 + jp) + 32],
                lhsT=lhsT, rhs=oh_t[:, q * 32:q * 32 + 32],
                start=(c == 0), stop=(c == CHUNKS - 1))
            if sg == 1 and jp == 3 and h == 1 and c == CHUNKS - 1:
                pphase(bank, st4["agg"])

        def pphase(bank, agg_ps):
            aggs = sbB.tile([4, 512], dt.float32, tag="aggs", name="aggs")
            nc.vector.tensor_copy(aggs[:], agg_ps[:])
            ptile = sb.tile([128, 4 * B], dt.float16, tag="ptile", name="ptile")
            nc.sync.dma_start(ptile[:], ptab_d[bank])
            trp = psT.tile([128, 512], dt.float32, tag="trp", name="trp")
            for j in range(4):
                nc.tensor.transpose(trp[:, 4 * j:4 * j + 4],
                                    aggs[:, 128 * j:128 * j + 128], ident[0:4, 0:4])
            s_trp = sbB.tile([128, 16], dt.float16, tag="strp4", name="strp4")
            nc.vector.tensor_copy(s_trp[:], trp[:, 0:16])
            for j in range(4):
                nc.tensor.matmul(pool_ps[:],
                                 lhsT=ptile[:, B * j:B * j + B],
                                 rhs=s_trp[:, 4 * j:4 * j + 4],
                                 start=(bank == 0 and j == 0),
                                 stop=(bank == BANKS - 1 and j == 3))

        agg_phase_split(table, 0, BANKS, 4, body4)

        pooled = sbB.tile([B, 4], dt.float32, name="pooled")
        nc.vector.tensor_copy(pooled[:], pool_ps[:])
        poolT_ps = psC.tile([4, 512], dt.float32, tag="z4", name="poolT_ps")
        nc.tensor.transpose(poolT_ps[:, 0:B], pooled[:], ident[0:B, 0:B])
        pooledT = sbB.tile([4, B], dt.float32, name="pooledT")
        nc.vector.tensor_copy(pooledT[:], poolT_ps[:, 0:B])
        nc.sync.dma_start(pool_in[:], pooledT[:])
        if os.environ.get("GCN_NO_CC"):
            nc.sync.dma_start(pool_out[:], pool_in[:])
        else:
            nc.gpsimd.collective_compute(
                "AllReduce", AG.add, replica_groups=[list(range(NC))],
                ins=[pool_in[:].opt()], outs=[pool_out[:].opt()])
        res = sbB.tile([4, B], dt.float32, name="res")
        nc.sync.dma_start(res[:], pool_out[:])
        res2 = sbB.tile([4, B], dt.float32, name="res2")
        nc.vector.tensor_scalar(res2[:], res[:], cvt[:], None, AG.add)
        nc.sync.dma_start(out_d[:], res2[:])

    nc.compile()
    return nc


def build(inputs):
    """Host preprocessing + program build. Returns (nc, in_maps)."""
    x = np.asarray(inputs["x"], F32)
    edge_index = np.asarray(inputs["edge_index"])
    batch = np.asarray(inputs["batch"])
    W = [np.asarray(inputs[f"W{i}"], F32) for i in range(1, 5)]
    b = [np.asarray(inputs[f"b{i}"], F32) for i in range(1, 5)]
    a = [np.asarray(inputs[f"a{i}"], F32) for i in range(1, 4)]
    lw1 = np.asarray(inputs["lw1"], F32)
    lb1 = np.asarray(inputs["lb1"], F32)
    lw2 = np.asarray(inputs["lw2"], F32)
    lb2 = np.asarray(inputs["lb2"], F32)

    cfg, xtab, idx123, oh123, idxS, ohS, pooltab = _preprocess(x, edge_index, batch)

    W4p = (W[3] @ lw1 @ lw2).astype(F32)                     # [64, 4]
    cv = (b[3] @ lw1 @ lw2 + lb1 @ lw2 + lb2).astype(F32)    # [4]

    nc = _build_program(cfg)

    in_maps = []
    for c in range(NC):
        m = dict(
            xtab=xtab, idx123=idx123[c], oh123=oh123[c],
            idxS=idxS[c], ohS=ohS[c], pooltab=pooltab[c],
            W1=W[0], W2=W[1], W3=W[2], W4=W4p,
            b1=b[0].reshape(-1, 1), b2=b[1].reshape(-1, 1), b3=b[2].reshape(-1, 1),
            a1=np.full((8, 1), a[0][0], F32),
            a2=np.full((32, 1), a[1][0], F32),
            a3=np.full((64, 1), a[2][0], F32),
            cvec=cv.reshape(4, 1),
        )
        in_maps.append(m)
    return nc, in_maps


def kernel(**inputs):
    nc, in_maps = build(inputs)
    from concourse.bass_utils import run_bass_kernel_spmd
    res = run_bass_kernel_spmd(nc, in_maps, list(range(NC)))
    outT = res.results[0]["out"]      # [4, B]
    return np.ascontiguousarray(outT.T.astype(F32))          # [B, 4]

